# revision 29
# baseline (speedup 1.0000x reference)
"""DIEN (GRU + AUGRU + MLP) Trainium2 Bass kernel, data-parallel over batch on 8 NeuronCores.

Hardcoded problem shape: B=4096, T=200, E=H=128, V=1e6 (see harness spec).
Layout on device: transposed [feature(=partition), batch] everywhere.

Host-side preprocessing:
  - weights pre-transposed into matmul lhsT layout ([K, M] with K = contraction dim)
  - AUGRU (layer-2) z-gate weight rows pre-negated so sigmoid directly yields (1-z)
    [h' = h + a*rg*(1-z)*(n-h)]
"""

import os
import numpy as np

B, T, E, H, V = 4096, 200, 128, 128, 1000000
NCORES = 8
BC = B // NCORES           # 512 batch per core
P = 128
NCHUNK = 2                 # batch sub-chunks for cross-engine pipelining
CW = BC // NCHUNK          # chunk width (256)

# compute dtype for SBUF-resident tensors fed to matmuls / elementwise
# (PSUM accumulation always fp32). fp16: full PE speed + DVE 2x packing like
# bf16, but 10-bit mantissa -> ~8x better elementwise precision.
CDT = os.environ.get("DIEN_DTYPE", "fp16")
assert CDT in ("fp16", "bf16", "fp32")


def _build(nc, Tsteps):
    import concourse.bass as bass
    import concourse.mybir as mybir
    import concourse.tile as tile
    from concourse.masks import make_identity

    dt = mybir.dt
    f32 = dt.float32
    cdt = {"fp16": dt.float16, "bf16": dt.bfloat16, "fp32": dt.float32}[CDT]
    AF = mybir.ActivationFunctionType

    # ---------------- DRAM I/O ----------------
    emb_d = nc.dram_tensor("emb", [V, E], cdt, kind="ExternalInput")
    uh_d = nc.dram_tensor("user_hist", [BC, Tsteps], dt.int32, kind="ExternalInput")
    ad_d = nc.dram_tensor("ad_feature", [BC, 1], dt.int32, kind="ExternalInput")
    wih1_d = nc.dram_tensor("wih1T", [E, 3 * H], cdt, kind="ExternalInput")
    whh1_d = nc.dram_tensor("whh1T", [H, 3 * H], cdt, kind="ExternalInput")
    wih2_d = nc.dram_tensor("wih2T", [E, 3 * H], cdt, kind="ExternalInput")
    whh2_d = nc.dram_tensor("whh2T", [H, 3 * H], cdt, kind="ExternalInput")
    wa_d = nc.dram_tensor("waT", [H, 1], cdt, kind="ExternalInput")
    wgbc_d = nc.dram_tensor("wgbc", [H, P], cdt, kind="ExternalInput")
    w1_d = nc.dram_tensor("w1T", [H, 64], f32, kind="ExternalInput")
    w2_d = nc.dram_tensor("w2T", [64, 32], f32, kind="ExternalInput")
    w3_d = nc.dram_tensor("w3T", [32, 1], f32, kind="ExternalInput")
    b1_d = nc.dram_tensor("b1", [64, 1], f32, kind="ExternalInput")
    b2_d = nc.dram_tensor("b2", [32, 1], f32, kind="ExternalInput")
    b3_d = nc.dram_tensor("b3", [1, 1], f32, kind="ExternalInput")
    # scalars folded on host into these: bg (AUGRU gate bias), ba*g handled via g row
    bg_d = nc.dram_tensor("bg", [1, 1], f32, kind="ExternalInput")

    out_d = nc.dram_tensor("out", [1, BC], f32, kind="ExternalOutput")

    # DRAM scratch
    xT_d = nc.dram_tensor("xT_scratch", [Tsteps, E, BC], cdt, kind="Internal")
    sc_d = nc.dram_tensor("sc_scratch", [Tsteps, 1, BC], f32, kind="Internal")
    at_d = nc.dram_tensor("at_scratch", [Tsteps, 1, BC], cdt, kind="Internal")

    with tile.TileContext(nc) as tc:
        with (
            tc.tile_pool(name="const", bufs=1) as cp,
            tc.tile_pool(name="gat", bufs=3) as gp,
            tc.tile_pool(name="xt", bufs=3) as xp,
            tc.tile_pool(name="hh", bufs=2) as hp,
            tc.tile_pool(name="gates", bufs=2) as ep,
            tc.tile_pool(name="small", bufs=3) as sp,
            tc.tile_pool(name="ps_rz", bufs=2, space="PSUM") as ps_rz,
            tc.tile_pool(name="ps_ih", bufs=1, space="PSUM") as ps_ih,
            # rz tiles span 2 banks: [r(512) | z(512)]; ih tiles: [in(512) | hn(512)]
            tc.tile_pool(name="ps_t", bufs=1, space="PSUM") as ps_t,
            tc.tile_pool(name="ps_s", bufs=1, space="PSUM") as ps_s,
        ):
            # ---------------- constants / weights ----------------
            ident = cp.tile([P, P], cdt)
            make_identity(nc, ident[:])
            wih1 = cp.tile([E, 3 * H], cdt)
            whh1 = cp.tile([H, 3 * H], cdt)
            wih2 = cp.tile([E, 3 * H], cdt)
            whh2 = cp.tile([H, 3 * H], cdt)
            wa = cp.tile([H, 1], cdt)
            wgbc = cp.tile([H, P], cdt)
            for sb_t, dr in ((wih1, wih1_d), (whh1, whh1_d), (wih2, wih2_d),
                             (whh2, whh2_d), (wa, wa_d), (wgbc, wgbc_d)):
                nc.sync.dma_start(sb_t[:], dr[:])
            w1 = cp.tile([H, 64], f32)
            w2 = cp.tile([64, 32], f32)
            w3 = cp.tile([32, 1], f32)
            b1 = cp.tile([64, 1], f32)
            b2 = cp.tile([32, 1], f32)
            b3 = cp.tile([1, 1], f32)
            bg = cp.tile([1, 1], f32)
            for sb_t, dr in ((w1, w1_d), (w2, w2_d), (w3, w3_d),
                             (b1, b1_d), (b2, b2_d), (b3, b3_d), (bg, bg_d)):
                nc.sync.dma_start(sb_t[:], dr[:])
            ones_col = cp.tile([P, 1], f32)       # lhsT for partition-sum (f32 rhs)
            nc.gpsimd.memset(ones_col[:], 1.0)
            ones_row = cp.tile([1, P], f32)       # lhsT for partition-broadcast (f32 rhs)
            nc.gpsimd.memset(ones_row[:], 1.0)
            ones_row_c = cp.tile([1, P], cdt)     # lhsT for partition-broadcast (cdt rhs)
            nc.gpsimd.memset(ones_row_c[:], 1.0)
            ones_col_c = cp.tile([P, 1], cdt)     # lhsT for partition-sum (cdt rhs)
            nc.gpsimd.memset(ones_col_c[:], 1.0)

            # user history indices: partition = b % 128, free = [chunk(4), t]
            uh = cp.tile([P, Tsteps, 4], dt.int32)
            uh_v = uh_d[:].rearrange("(c p) t -> p t c", p=P)
            nc.sync.dma_start(uh[:], uh_v)

            # ---------------- phase 0: ad embedding -> g row ----------------
            adidx = cp.tile([P, 4], dt.int32)
            nc.sync.dma_start(adidx[:], ad_d[:].rearrange("(c p) o -> p (c o)", p=P))
            adg = gp.tile([P, 4, E], cdt, tag="gath")
            for c in range(4):
                nc.gpsimd.indirect_dma_start(
                    out=adg[:, c, :], out_offset=None, in_=emb_d[:],
                    in_offset=bass.IndirectOffsetOnAxis(ap=adidx[:, c:c + 1], axis=0))
            adT_ps = ps_t.tile([P, BC], cdt, tag="tps")
            for c in range(4):
                nc.tensor.transpose(out=adT_ps[:, c * P:(c + 1) * P],
                                    in_=adg[:, c, :], identity=ident[:])
            adT = xp.tile([E, BC], cdt, tag="xT")
            nc.vector.tensor_copy(adT[:], adT_ps[:])
            g_ps = ps_s.tile([1, BC], f32, tag="sps")
            nc.tensor.matmul(g_ps[:], ones_col_c[:], adT[:], start=True, stop=True)
            g_row = cp.tile([1, BC], f32)
            nc.vector.tensor_copy(g_row[:], g_ps[:])

            # ---------------- hidden state ----------------
            h = hp.tile([H, BC], cdt, tag="h")
            nc.gpsimd.memset(h[:], 0.0)

            def gru_gates(xT, h_prev, wih, whh, layer):
                """Gate matmuls + elementwise for one step.

                PSUM: one [P, 1024] tile per gate-pair spanning two banks:
                rz = [r | z], ih = [i_n | h_n]; each N=512 matmul targets one
                bank, so each bank is a clean accumulation group. Input-side
                (gi) matmuls are emitted first: they depend only on xT so the
                PE can run them into the double-buffered rz banks while the
                previous step's elementwise chain drains.
                """
                p_rz = ps_rz.tile([P, 2 * BC], f32, tag="rz")
                p_ih = ps_ih.tile([P, 2 * BC], f32, tag="ih")
                nc.tensor.matmul(p_rz[:, 0:BC], wih[:, 0:H], xT[:],
                                 start=True, stop=False)
                nc.tensor.matmul(p_rz[:, BC:2 * BC], wih[:, H:2 * H], xT[:],
                                 start=True, stop=False)
                nc.tensor.matmul(p_ih[:, 0:BC], wih[:, 2 * H:3 * H], xT[:],
                                 start=True, stop=True)
                nc.tensor.matmul(p_rz[:, 0:BC], whh[:, 0:H], h_prev[:],
                                 start=False, stop=True)
                nc.tensor.matmul(p_rz[:, BC:2 * BC], whh[:, H:2 * H], h_prev[:],
                                 start=False, stop=True)
                nc.tensor.matmul(p_ih[:, BC:2 * BC], whh[:, 2 * H:3 * H], h_prev[:],
                                 start=True, stop=True)
                res = []
                for ch in range(NCHUNK):
                    cs = slice(ch * CW, (ch + 1) * CW)
                    rz = ep.tile([P, 2 * CW], cdt, tag=f"g_rz{ch}")
                    # [r_ch | z_ch]: 2D free AP over the two banks of p_rz
                    nc.scalar.activation(
                        rz[:].rearrange("p (g w) -> p g w", g=2),
                        p_rz[:].rearrange("p (g b) -> p g b", g=2)[:, :, cs],
                        AF.Sigmoid)
                    m = ep.tile([P, CW], cdt, tag=f"g_m{ch}")
                    nc.vector.tensor_mul(m[:], rz[:, 0:CW], p_ih[:, BC + ch * CW:BC + (ch + 1) * CW])
                    npre = ep.tile([P, CW], cdt, tag=f"g_np{ch}")
                    nc.vector.tensor_add(npre[:], m[:], p_ih[:, ch * CW:(ch + 1) * CW])
                    n_t = ep.tile([P, CW], cdt, tag=f"g_n{ch}")
                    nc.scalar.activation(n_t[:], npre[:], AF.Tanh)
                    res.append((n_t, rz))
                return res

            # =================== pass A: gather + GRU1 + scores ===================
            for t in range(Tsteps):
                gat = gp.tile([P, 4, E], cdt, tag="gath")
                for c in range(4):
                    nc.gpsimd.indirect_dma_start(
                        out=gat[:, c, :], out_offset=None, in_=emb_d[:],
                        in_offset=bass.IndirectOffsetOnAxis(ap=uh[:, t, c:c + 1], axis=0))
                xt_ps = ps_t.tile([P, BC], cdt, tag="tps")
                for c in range(4):
                    nc.tensor.transpose(
                        out=xt_ps[:, c * P:(c + 1) * P],
                        in_=gat[:, c, :], identity=ident[:])
                xT = xp.tile([E, BC], cdt, tag="xT")
                nc.scalar.copy(xT[:], xt_ps[:])
                nc.sync.dma_start(xT_d[t], xT[:])

                h_new = hp.tile([H, BC], cdt, tag="h")
                for ch, (n_t, rz) in enumerate(gru_gates(xT, h, wih1, whh1, 1)):
                    cs = slice(ch * CW, (ch + 1) * CW)
                    s_t = ep.tile([P, CW], cdt, tag=f"g_s{ch}")
                    nc.vector.tensor_sub(s_t[:], h[:, cs], n_t[:])
                    u_t = ep.tile([P, CW], cdt, tag=f"g_u{ch}")
                    nc.vector.tensor_mul(u_t[:], rz[:, CW:2 * CW], s_t[:])
                    nc.vector.tensor_add(h_new[:, cs], n_t[:], u_t[:])
                h = h_new

                sc_ps = ps_s.tile([1, BC], f32, tag="sps")
                nc.tensor.matmul(sc_ps[:], wa[:], h[:], start=True, stop=True)
                sc = sp.tile([1, BC], f32, tag="sc")
                nc.vector.tensor_copy(sc[:], sc_ps[:])
                nc.sync.dma_start(sc_d[t], sc[:])

            # =================== softmax over t (scaled by g) ===================
            TT0 = min(P, Tsteps)
            TT1 = Tsteps - TT0
            gb_ps = ps_t.tile([P, BC], f32, tag="tps")
            nc.tensor.matmul(gb_ps[:], ones_row[:], g_row[:], start=True, stop=True)
            gb = xp.tile([P, BC], f32, tag="xT")
            nc.vector.tensor_copy(gb[:], gb_ps[:])
            den_ps = ps_s.tile([1, BC], f32, tag="sps")
            ex_tiles = []
            for i, (t0, tl) in enumerate(((0, TT0), (TT0, TT1))):
                if tl == 0:
                    continue
                s_sb = ep.tile([P, BC], f32, tag=f"sm{i}")
                nc.sync.dma_start(s_sb[:tl, :],
                                  sc_d[t0:t0 + tl].rearrange("t o b -> (t o) b"))
                sg = ep.tile([P, BC], f32, tag=f"smg{i}")
                nc.vector.tensor_mul(sg[:tl, :], s_sb[:tl, :], gb[:tl, :])
                exp_t = ep.tile([P, BC], f32, tag=f"sme{i}")
                nc.scalar.activation(exp_t[:tl, :], sg[:tl, :], AF.Exp)
                nc.tensor.matmul(den_ps[:], ones_col[:tl, :], exp_t[:tl, :],
                                 start=(i == 0), stop=(tl + t0 == Tsteps))
                ex_tiles.append((exp_t, t0, tl))
            den = sp.tile([1, BC], f32, tag="sc")
            nc.vector.tensor_copy(den[:], den_ps[:])
            rden = sp.tile([1, BC], f32, tag="rden")
            nc.vector.reciprocal(rden[:], den[:])
            rb_ps = ps_t.tile([P, BC], f32, tag="tps")
            nc.tensor.matmul(rb_ps[:], ones_row[:], rden[:], start=True, stop=True)
            rb = xp.tile([P, BC], f32, tag="xT")
            nc.vector.tensor_copy(rb[:], rb_ps[:])
            for exp_t, t0, tl in ex_tiles:
                at_sb = ep.tile([P, BC], cdt, tag="smA")
                nc.vector.tensor_mul(at_sb[:tl, :], exp_t[:tl, :], rb[:tl, :])
                nc.sync.dma_start(at_d[t0:t0 + tl].rearrange("t o b -> (t o) b"),
                                  at_sb[:tl, :])

            # =================== pass B: AUGRU ===================
            for t in range(Tsteps):
                xT = xp.tile([E, BC], cdt, tag="xT")
                nc.sync.dma_start(xT[:], xT_d[t])
                a_sb = sp.tile([1, BC], cdt, tag="a_t")
                nc.sync.dma_start(a_sb[:], at_d[t])

                # rg = sigmoid(wg . h), broadcast over partitions via matmul
                rg_ps = ps_t.tile([P, BC], f32, tag="tps")
                nc.tensor.matmul(rg_ps[:], wgbc[:], h[:], start=True, stop=True)
                rg = ep.tile([P, BC], cdt, tag="rg")
                nc.scalar.activation(rg[:], rg_ps[:], AF.Sigmoid)
                # a_t broadcast over partitions: ones (K=1) matmul
                ab_ps = ps_s.tile([P, BC], f32, tag="sps")
                nc.tensor.matmul(ab_ps[:], ones_row_c[:], a_sb[:], start=True, stop=True)
                c_sb = ep.tile([P, BC], cdt, tag="c")
                nc.vector.tensor_mul(c_sb[:], rg[:], ab_ps[:])

                h_new = hp.tile([H, BC], cdt, tag="h")
                for ch, (n_t, rz) in enumerate(gru_gates(xT, h, wih2, whh2, 2)):
                    cs = slice(ch * CW, (ch + 1) * CW)
                    d_t = ep.tile([P, CW], cdt, tag=f"g_s{ch}")
                    nc.vector.tensor_sub(d_t[:], n_t[:], h[:, cs])       # n - h
                    e_t = ep.tile([P, CW], cdt, tag=f"g_u{ch}")
                    nc.vector.tensor_mul(e_t[:], rz[:, CW:2 * CW], d_t[:])  # (1-z)(n-h)
                    f_t = ep.tile([P, CW], cdt, tag=f"g_f{ch}")
                    nc.vector.tensor_mul(f_t[:], c_sb[:, cs], e_t[:])
                    nc.vector.tensor_add(h_new[:, cs], h[:, cs], f_t[:])
                h = h_new

            # =================== MLP head ===================
            hf = ep.tile([H, BC], f32, tag="hf")
            nc.vector.tensor_copy(hf[:], h[:])
            x1_ps = ps_t.tile([64, BC], f32, tag="tps")
            nc.tensor.matmul(x1_ps[:], w1[:], hf[:], start=True, stop=True)
            x1 = ep.tile([64, BC], f32, tag="mlp1")
            nc.scalar.activation(x1[:], x1_ps[:], AF.Relu, bias=b1[:, 0:1])
            x2_ps = ps_t.tile([32, BC], f32, tag="tps")
            nc.tensor.matmul(x2_ps[:], w2[:], x1[:], start=True, stop=True)
            x2 = ep.tile([32, BC], f32, tag="mlp2")
            nc.scalar.activation(x2[:], x2_ps[:], AF.Relu, bias=b2[:, 0:1])
            y_ps = ps_s.tile([1, BC], f32, tag="sps")
            nc.tensor.matmul(y_ps[:], w3[:], x2[:], start=True, stop=True)
            y = sp.tile([1, BC], f32, tag="y")
            nc.scalar.activation(y[:], y_ps[:], AF.Identity, bias=b3[:, 0:1])
            nc.sync.dma_start(out_d[:], y[:])

    return nc


def _prep_inputs(user_hist, ad_feature, emb, Wih1, Whh1, bih1, bhh1, wa, ba,
                 Wih2, Whh2, bih2, bhh2, wg, bg, W1, b1, W2, b2, W3, b3,
                 Tsteps):
    """Host-side preprocessing + sharding. Returns list of per-core input dicts."""
    import ml_dtypes
    np_cdt = {"fp16": np.float16, "bf16": ml_dtypes.bfloat16, "fp32": np.float32}[CDT]

    f32 = np.float32
    # all GRU biases must be zero for the fast path (true for this problem)
    assert not (np.any(bih1) or np.any(bhh1) or np.any(bih2) or np.any(bhh2)), \
        "nonzero GRU biases not supported by this kernel build"

    def gate_lhsT(W, negate_z):
        # W: [3H, X] torch layout (r,z,n) -> lhsT [X, 3H]
        Wt = np.ascontiguousarray(W.T).astype(f32)
        if negate_z:
            Wt = Wt.copy()
            Wt[:, H:2 * H] *= -1.0
        return Wt.astype(np_cdt)

    common = {
        "emb": np.ascontiguousarray(emb).astype(np_cdt),
        "wih1T": gate_lhsT(Wih1, False),
        "whh1T": gate_lhsT(Whh1, False),
        "wih2T": gate_lhsT(Wih2, True),
        "whh2T": gate_lhsT(Whh2, True),
        "waT": np.ascontiguousarray(wa.reshape(H, 1), dtype=f32).astype(np_cdt),
        "wgbc": np.ascontiguousarray(np.tile(wg.reshape(H, 1), (1, P)), dtype=f32).astype(np_cdt),
        "w1T": np.ascontiguousarray(W1.T, dtype=f32),
        "w2T": np.ascontiguousarray(W2.T, dtype=f32),
        "w3T": np.ascontiguousarray(W3.T, dtype=f32),
        "b1": np.ascontiguousarray(b1.reshape(64, 1), dtype=f32),
        "b2": np.ascontiguousarray(b2.reshape(32, 1), dtype=f32),
        "b3": np.ascontiguousarray(b3.reshape(1, 1), dtype=f32),
        "bg": np.asarray(bg, dtype=f32).reshape(1, 1),
    }
    # ba shifts all scores by a constant; softmax(s*g) with s+ba needs exp((s+ba)*g).
    # ba == 0 in this problem; assert to be safe.
    assert float(np.asarray(ba)) == 0.0, "nonzero attention bias not supported"
    assert float(np.asarray(bg)) == 0.0, "nonzero AUGRU gate bias not supported"

    in_maps = []
    for c in range(NCORES):
        rows = slice(c * BC, (c + 1) * BC)
        m = dict(common)
        m["user_hist"] = np.ascontiguousarray(user_hist[rows, :Tsteps], dtype=np.int32)
        m["ad_feature"] = np.ascontiguousarray(
            ad_feature[rows].reshape(BC, 1), dtype=np.int32)
        in_maps.append(m)
    return in_maps


_CACHE = {}


def kernel(user_hist, ad_feature, emb, Wih1, Whh1, bih1, bhh1, wa, ba,
           Wih2, Whh2, bih2, bhh2, wg, bg, W1, b1, W2, b2, W3, b3,
           _trace=False, _tsteps=None):
    import concourse.bacc as bacc
    from concourse.bass_utils import run_bass_kernel_spmd

    Tsteps = _tsteps or T
    key = Tsteps
    if key not in _CACHE:
        nc = bacc.Bacc("TRN2", num_devices=1, enable_asserts=True)
        _build(nc, Tsteps)
        nc.compile()
        _CACHE[key] = nc
    nc = _CACHE[key]

    in_maps = _prep_inputs(user_hist, ad_feature, emb, Wih1, Whh1, bih1, bhh1,
                           wa, ba, Wih2, Whh2, bih2, bhh2, wg, bg,
                           W1, b1, W2, b2, W3, b3, Tsteps)
    r = run_bass_kernel_spmd(nc, in_maps, core_ids=list(range(NCORES)),
                             trace=_trace)
    out = np.concatenate(
        [np.asarray(r.results[c]["out"]).reshape(BC, 1) for c in range(NCORES)],
        axis=0)
    if _trace:
        kernel._last_result = r
    return out.astype(np.float32)


# revision 30
# speedup vs baseline: 1.0445x; 1.0445x over previous
"""DIEN (GRU + AUGRU + MLP) Trainium2 Bass kernel, data-parallel over batch on 8 NeuronCores.

Hardcoded problem shape: B=4096, T=200, E=H=128, V=1e6 (see harness spec).
Layout on device: transposed [feature(=partition), batch] everywhere.

Host-side preprocessing:
  - weights pre-transposed into matmul lhsT layout ([K, M] with K = contraction dim)
  - AUGRU (layer-2) z-gate weight rows pre-negated so sigmoid directly yields (1-z)
    [h' = h + a*rg*(1-z)*(n-h)]
"""

import os
import numpy as np

B, T, E, H, V = 4096, 200, 128, 128, 1000000
NCORES = 8
BC = B // NCORES           # 512 batch per core
P = 128
NCHUNK = 2                 # batch sub-chunks for cross-engine pipelining
CW = BC // NCHUNK          # chunk width (256)

# compute dtype for SBUF-resident tensors fed to matmuls / elementwise
# (PSUM accumulation always fp32). fp16: full PE speed + DVE 2x packing like
# bf16, but 10-bit mantissa -> ~8x better elementwise precision.
CDT = os.environ.get("DIEN_DTYPE", "fp16")
assert CDT in ("fp16", "bf16", "fp32")


def _build(nc, Tsteps):
    import concourse.bass as bass
    import concourse.mybir as mybir
    import concourse.tile as tile
    from concourse.masks import make_identity

    dt = mybir.dt
    f32 = dt.float32
    cdt = {"fp16": dt.float16, "bf16": dt.bfloat16, "fp32": dt.float32}[CDT]
    AF = mybir.ActivationFunctionType

    # ---------------- DRAM I/O ----------------
    emb_d = nc.dram_tensor("emb", [V, E], cdt, kind="ExternalInput")
    uh_d = nc.dram_tensor("user_hist", [BC, Tsteps], dt.int32, kind="ExternalInput")
    ad_d = nc.dram_tensor("ad_feature", [BC, 1], dt.int32, kind="ExternalInput")
    wih1_d = nc.dram_tensor("wih1T", [E, 3 * H], cdt, kind="ExternalInput")
    whh1_d = nc.dram_tensor("whh1T", [H, 3 * H], cdt, kind="ExternalInput")
    wih2_d = nc.dram_tensor("wih2T", [E, 3 * H], cdt, kind="ExternalInput")
    whh2_d = nc.dram_tensor("whh2T", [H, 3 * H], cdt, kind="ExternalInput")
    wa_d = nc.dram_tensor("waT", [H, 1], cdt, kind="ExternalInput")
    wgbc_d = nc.dram_tensor("wgbc", [H, P], cdt, kind="ExternalInput")
    w1_d = nc.dram_tensor("w1T", [H, 64], f32, kind="ExternalInput")
    w2_d = nc.dram_tensor("w2T", [64, 32], f32, kind="ExternalInput")
    w3_d = nc.dram_tensor("w3T", [32, 1], f32, kind="ExternalInput")
    b1_d = nc.dram_tensor("b1", [64, 1], f32, kind="ExternalInput")
    b2_d = nc.dram_tensor("b2", [32, 1], f32, kind="ExternalInput")
    b3_d = nc.dram_tensor("b3", [1, 1], f32, kind="ExternalInput")
    # scalars folded on host into these: bg (AUGRU gate bias), ba*g handled via g row
    bg_d = nc.dram_tensor("bg", [1, 1], f32, kind="ExternalInput")

    out_d = nc.dram_tensor("out", [1, BC], f32, kind="ExternalOutput")

    # DRAM scratch
    xT_d = nc.dram_tensor("xT_scratch", [Tsteps, E, BC], cdt, kind="Internal")
    sc_d = nc.dram_tensor("sc_scratch", [Tsteps, 1, BC], f32, kind="Internal")
    at_d = nc.dram_tensor("at_scratch", [Tsteps, 1, BC], cdt, kind="Internal")

    with tile.TileContext(nc) as tc:
        with (
            tc.tile_pool(name="const", bufs=1) as cp,
            tc.tile_pool(name="gat", bufs=3) as gp,
            tc.tile_pool(name="xt", bufs=3) as xp,
            tc.tile_pool(name="hh", bufs=2) as hp,
            tc.tile_pool(name="gates", bufs=2) as ep,
            tc.tile_pool(name="small", bufs=3) as sp,
            tc.tile_pool(name="ps_rz", bufs=2, space="PSUM") as ps_rz,
            tc.tile_pool(name="ps_ih", bufs=1, space="PSUM") as ps_ih,
            # rz tiles span 2 banks: [r(512) | z(512)]; ih tiles: [in(512) | hn(512)]
            tc.tile_pool(name="ps_t", bufs=1, space="PSUM") as ps_t,
            tc.tile_pool(name="ps_s", bufs=1, space="PSUM") as ps_s,
        ):
            # ---------------- constants / weights ----------------
            ident = cp.tile([P, P], cdt)
            make_identity(nc, ident[:])
            wih1 = cp.tile([E, 3 * H], cdt)
            whh1 = cp.tile([H, 3 * H], cdt)
            wih2 = cp.tile([E, 3 * H], cdt)
            whh2 = cp.tile([H, 3 * H], cdt)
            wa = cp.tile([H, 1], cdt)
            wgbc = cp.tile([H, P], cdt)
            for sb_t, dr in ((wih1, wih1_d), (whh1, whh1_d), (wih2, wih2_d),
                             (whh2, whh2_d), (wa, wa_d), (wgbc, wgbc_d)):
                nc.sync.dma_start(sb_t[:], dr[:])
            w1 = cp.tile([H, 64], f32)
            w2 = cp.tile([64, 32], f32)
            w3 = cp.tile([32, 1], f32)
            b1 = cp.tile([64, 1], f32)
            b2 = cp.tile([32, 1], f32)
            b3 = cp.tile([1, 1], f32)
            bg = cp.tile([1, 1], f32)
            for sb_t, dr in ((w1, w1_d), (w2, w2_d), (w3, w3_d),
                             (b1, b1_d), (b2, b2_d), (b3, b3_d), (bg, bg_d)):
                nc.sync.dma_start(sb_t[:], dr[:])
            ones_col = cp.tile([P, 1], f32)       # lhsT for partition-sum (f32 rhs)
            nc.gpsimd.memset(ones_col[:], 1.0)
            ones_row = cp.tile([1, P], f32)       # lhsT for partition-broadcast (f32 rhs)
            nc.gpsimd.memset(ones_row[:], 1.0)
            ones_row_c = cp.tile([1, P], cdt)     # lhsT for partition-broadcast (cdt rhs)
            nc.gpsimd.memset(ones_row_c[:], 1.0)
            ones_col_c = cp.tile([P, 1], cdt)     # lhsT for partition-sum (cdt rhs)
            nc.gpsimd.memset(ones_col_c[:], 1.0)

            # user history indices: partition = b % 128, free = [chunk(4), t]
            uh = cp.tile([P, Tsteps, 4], dt.int32)
            uh_v = uh_d[:].rearrange("(c p) t -> p t c", p=P)
            nc.sync.dma_start(uh[:], uh_v)

            # ---------------- phase 0: ad embedding -> g row ----------------
            adidx = cp.tile([P, 4], dt.int32)
            nc.sync.dma_start(adidx[:], ad_d[:].rearrange("(c p) o -> p (c o)", p=P))
            adg = gp.tile([P, 4, E], cdt, tag="gath")
            for c in range(4):
                nc.gpsimd.indirect_dma_start(
                    out=adg[:, c, :], out_offset=None, in_=emb_d[:],
                    in_offset=bass.IndirectOffsetOnAxis(ap=adidx[:, c:c + 1], axis=0))
            adT_ps = ps_t.tile([P, BC], cdt, tag="tps")
            for c in range(4):
                nc.tensor.transpose(out=adT_ps[:, c * P:(c + 1) * P],
                                    in_=adg[:, c, :], identity=ident[:])
            adT = xp.tile([E, BC], cdt, tag="xT")
            nc.vector.tensor_copy(adT[:], adT_ps[:])
            g_ps = ps_s.tile([1, BC], f32, tag="sps")
            nc.tensor.matmul(g_ps[:], ones_col_c[:], adT[:], start=True, stop=True)
            g_row = cp.tile([1, BC], f32)
            nc.vector.tensor_copy(g_row[:], g_ps[:])

            # ---------------- hidden state ----------------
            h = hp.tile([H, BC], cdt, tag="h")
            nc.gpsimd.memset(h[:], 0.0)

            def gru_gates(xT, h_prev, wih, whh, layer):
                """Gate matmuls + elementwise for one step.

                PSUM: one [P, 1024] tile per gate-pair spanning two banks:
                rz = [r | z], ih = [i_n | h_n]; each N=512 matmul targets one
                bank, so each bank is a clean accumulation group. Input-side
                (gi) matmuls are emitted first: they depend only on xT so the
                PE can run them into the double-buffered rz banks while the
                previous step's elementwise chain drains.
                """
                p_rz = ps_rz.tile([P, 2 * BC], f32, tag="rz")
                p_ih = ps_ih.tile([P, 2 * BC], f32, tag="ih")
                nc.tensor.matmul(p_rz[:, 0:BC], wih[:, 0:H], xT[:],
                                 start=True, stop=False)
                nc.tensor.matmul(p_rz[:, BC:2 * BC], wih[:, H:2 * H], xT[:],
                                 start=True, stop=False)
                nc.tensor.matmul(p_ih[:, 0:BC], wih[:, 2 * H:3 * H], xT[:],
                                 start=True, stop=True)
                # recurrent matmuls chunked so chunk A's sigma can start
                # while chunk B's h from the previous step is still blending
                for ch in range(NCHUNK):
                    cs = slice(ch * CW, (ch + 1) * CW)
                    nc.tensor.matmul(p_rz[:, ch * CW:(ch + 1) * CW],
                                     whh[:, 0:H], h_prev[:, cs],
                                     start=False, stop=(ch == NCHUNK - 1))
                    nc.tensor.matmul(p_rz[:, BC + ch * CW:BC + (ch + 1) * CW],
                                     whh[:, H:2 * H], h_prev[:, cs],
                                     start=False, stop=(ch == NCHUNK - 1))
                    nc.tensor.matmul(p_ih[:, BC + ch * CW:BC + (ch + 1) * CW],
                                     whh[:, 2 * H:3 * H], h_prev[:, cs],
                                     start=(ch == 0), stop=(ch == NCHUNK - 1))
                res = []
                for ch in range(NCHUNK):
                    cs = slice(ch * CW, (ch + 1) * CW)
                    rz = ep.tile([P, 2 * CW], cdt, tag=f"g_rz{ch}")
                    # [r_ch | z_ch]: 2D free AP over the two banks of p_rz
                    nc.scalar.activation(
                        rz[:].rearrange("p (g w) -> p g w", g=2),
                        p_rz[:].rearrange("p (g b) -> p g b", g=2)[:, :, cs],
                        AF.Sigmoid)
                    m = ep.tile([P, CW], cdt, tag=f"g_m{ch}")
                    nc.vector.tensor_mul(m[:], rz[:, 0:CW], p_ih[:, BC + ch * CW:BC + (ch + 1) * CW])
                    npre = ep.tile([P, CW], cdt, tag=f"g_np{ch}")
                    nc.vector.tensor_add(npre[:], m[:], p_ih[:, ch * CW:(ch + 1) * CW])
                    n_t = ep.tile([P, CW], cdt, tag=f"g_n{ch}")
                    nc.scalar.activation(n_t[:], npre[:], AF.Tanh)
                    res.append((n_t, rz))
                return res

            # =================== pass A: gather + GRU1 + scores ===================
            for t in range(Tsteps):
                gat = gp.tile([P, 4, E], cdt, tag="gath")
                for c in range(4):
                    nc.gpsimd.indirect_dma_start(
                        out=gat[:, c, :], out_offset=None, in_=emb_d[:],
                        in_offset=bass.IndirectOffsetOnAxis(ap=uh[:, t, c:c + 1], axis=0))
                xt_ps = ps_t.tile([P, BC], cdt, tag="tps")
                for c in range(4):
                    nc.tensor.transpose(
                        out=xt_ps[:, c * P:(c + 1) * P],
                        in_=gat[:, c, :], identity=ident[:])
                xT = xp.tile([E, BC], cdt, tag="xT")
                nc.scalar.copy(xT[:], xt_ps[:])
                nc.sync.dma_start(xT_d[t], xT[:])

                h_new = hp.tile([H, BC], cdt, tag="h")
                for ch, (n_t, rz) in enumerate(gru_gates(xT, h, wih1, whh1, 1)):
                    cs = slice(ch * CW, (ch + 1) * CW)
                    s_t = ep.tile([P, CW], cdt, tag=f"g_s{ch}")
                    nc.vector.tensor_sub(s_t[:], h[:, cs], n_t[:])
                    u_t = ep.tile([P, CW], cdt, tag=f"g_u{ch}")
                    nc.vector.tensor_mul(u_t[:], rz[:, CW:2 * CW], s_t[:])
                    nc.vector.tensor_add(h_new[:, cs], n_t[:], u_t[:])
                h = h_new

                sc_ps = ps_s.tile([1, BC], f32, tag="sps")
                nc.tensor.matmul(sc_ps[:], wa[:], h[:], start=True, stop=True)
                sc = sp.tile([1, BC], f32, tag="sc")
                nc.vector.tensor_copy(sc[:], sc_ps[:])
                nc.sync.dma_start(sc_d[t], sc[:])

            # =================== softmax over t (scaled by g) ===================
            TT0 = min(P, Tsteps)
            TT1 = Tsteps - TT0
            gb_ps = ps_t.tile([P, BC], f32, tag="tps")
            nc.tensor.matmul(gb_ps[:], ones_row[:], g_row[:], start=True, stop=True)
            gb = xp.tile([P, BC], f32, tag="xT")
            nc.vector.tensor_copy(gb[:], gb_ps[:])
            den_ps = ps_s.tile([1, BC], f32, tag="sps")
            ex_tiles = []
            for i, (t0, tl) in enumerate(((0, TT0), (TT0, TT1))):
                if tl == 0:
                    continue
                s_sb = ep.tile([P, BC], f32, tag=f"sm{i}")
                nc.sync.dma_start(s_sb[:tl, :],
                                  sc_d[t0:t0 + tl].rearrange("t o b -> (t o) b"))
                sg = ep.tile([P, BC], f32, tag=f"smg{i}")
                nc.vector.tensor_mul(sg[:tl, :], s_sb[:tl, :], gb[:tl, :])
                exp_t = ep.tile([P, BC], f32, tag=f"sme{i}")
                nc.scalar.activation(exp_t[:tl, :], sg[:tl, :], AF.Exp)
                nc.tensor.matmul(den_ps[:], ones_col[:tl, :], exp_t[:tl, :],
                                 start=(i == 0), stop=(tl + t0 == Tsteps))
                ex_tiles.append((exp_t, t0, tl))
            den = sp.tile([1, BC], f32, tag="sc")
            nc.vector.tensor_copy(den[:], den_ps[:])
            rden = sp.tile([1, BC], f32, tag="rden")
            nc.vector.reciprocal(rden[:], den[:])
            rb_ps = ps_t.tile([P, BC], f32, tag="tps")
            nc.tensor.matmul(rb_ps[:], ones_row[:], rden[:], start=True, stop=True)
            rb = xp.tile([P, BC], f32, tag="xT")
            nc.vector.tensor_copy(rb[:], rb_ps[:])
            for exp_t, t0, tl in ex_tiles:
                at_sb = ep.tile([P, BC], cdt, tag="smA")
                nc.vector.tensor_mul(at_sb[:tl, :], exp_t[:tl, :], rb[:tl, :])
                nc.sync.dma_start(at_d[t0:t0 + tl].rearrange("t o b -> (t o) b"),
                                  at_sb[:tl, :])

            # =================== pass B: AUGRU ===================
            for t in range(Tsteps):
                xT = xp.tile([E, BC], cdt, tag="xT")
                nc.sync.dma_start(xT[:], xT_d[t])
                a_sb = sp.tile([1, BC], cdt, tag="a_t")
                nc.sync.dma_start(a_sb[:], at_d[t])

                # rg = sigmoid(wg . h), broadcast over partitions via matmul
                rg_ps = ps_t.tile([P, BC], f32, tag="tps")
                nc.tensor.matmul(rg_ps[:], wgbc[:], h[:], start=True, stop=True)
                rg = ep.tile([P, BC], cdt, tag="rg")
                nc.scalar.activation(rg[:], rg_ps[:], AF.Sigmoid)
                # a_t broadcast over partitions: ones (K=1) matmul
                ab_ps = ps_s.tile([P, BC], f32, tag="sps")
                nc.tensor.matmul(ab_ps[:], ones_row_c[:], a_sb[:], start=True, stop=True)
                c_sb = ep.tile([P, BC], cdt, tag="c")
                nc.vector.tensor_mul(c_sb[:], rg[:], ab_ps[:])

                h_new = hp.tile([H, BC], cdt, tag="h")
                for ch, (n_t, rz) in enumerate(gru_gates(xT, h, wih2, whh2, 2)):
                    cs = slice(ch * CW, (ch + 1) * CW)
                    d_t = ep.tile([P, CW], cdt, tag=f"g_s{ch}")
                    nc.vector.tensor_sub(d_t[:], n_t[:], h[:, cs])       # n - h
                    e_t = ep.tile([P, CW], cdt, tag=f"g_u{ch}")
                    nc.vector.tensor_mul(e_t[:], rz[:, CW:2 * CW], d_t[:])  # (1-z)(n-h)
                    f_t = ep.tile([P, CW], cdt, tag=f"g_f{ch}")
                    nc.vector.tensor_mul(f_t[:], c_sb[:, cs], e_t[:])
                    nc.vector.tensor_add(h_new[:, cs], h[:, cs], f_t[:])
                h = h_new

            # =================== MLP head ===================
            hf = ep.tile([H, BC], f32, tag="hf")
            nc.vector.tensor_copy(hf[:], h[:])
            x1_ps = ps_t.tile([64, BC], f32, tag="tps")
            nc.tensor.matmul(x1_ps[:], w1[:], hf[:], start=True, stop=True)
            x1 = ep.tile([64, BC], f32, tag="mlp1")
            nc.scalar.activation(x1[:], x1_ps[:], AF.Relu, bias=b1[:, 0:1])
            x2_ps = ps_t.tile([32, BC], f32, tag="tps")
            nc.tensor.matmul(x2_ps[:], w2[:], x1[:], start=True, stop=True)
            x2 = ep.tile([32, BC], f32, tag="mlp2")
            nc.scalar.activation(x2[:], x2_ps[:], AF.Relu, bias=b2[:, 0:1])
            y_ps = ps_s.tile([1, BC], f32, tag="sps")
            nc.tensor.matmul(y_ps[:], w3[:], x2[:], start=True, stop=True)
            y = sp.tile([1, BC], f32, tag="y")
            nc.scalar.activation(y[:], y_ps[:], AF.Identity, bias=b3[:, 0:1])
            nc.sync.dma_start(out_d[:], y[:])

    return nc


def _prep_inputs(user_hist, ad_feature, emb, Wih1, Whh1, bih1, bhh1, wa, ba,
                 Wih2, Whh2, bih2, bhh2, wg, bg, W1, b1, W2, b2, W3, b3,
                 Tsteps):
    """Host-side preprocessing + sharding. Returns list of per-core input dicts."""
    import ml_dtypes
    np_cdt = {"fp16": np.float16, "bf16": ml_dtypes.bfloat16, "fp32": np.float32}[CDT]

    f32 = np.float32
    # all GRU biases must be zero for the fast path (true for this problem)
    assert not (np.any(bih1) or np.any(bhh1) or np.any(bih2) or np.any(bhh2)), \
        "nonzero GRU biases not supported by this kernel build"

    def gate_lhsT(W, negate_z):
        # W: [3H, X] torch layout (r,z,n) -> lhsT [X, 3H]
        Wt = np.ascontiguousarray(W.T).astype(f32)
        if negate_z:
            Wt = Wt.copy()
            Wt[:, H:2 * H] *= -1.0
        return Wt.astype(np_cdt)

    common = {
        "emb": np.ascontiguousarray(emb).astype(np_cdt),
        "wih1T": gate_lhsT(Wih1, False),
        "whh1T": gate_lhsT(Whh1, False),
        "wih2T": gate_lhsT(Wih2, True),
        "whh2T": gate_lhsT(Whh2, True),
        "waT": np.ascontiguousarray(wa.reshape(H, 1), dtype=f32).astype(np_cdt),
        "wgbc": np.ascontiguousarray(np.tile(wg.reshape(H, 1), (1, P)), dtype=f32).astype(np_cdt),
        "w1T": np.ascontiguousarray(W1.T, dtype=f32),
        "w2T": np.ascontiguousarray(W2.T, dtype=f32),
        "w3T": np.ascontiguousarray(W3.T, dtype=f32),
        "b1": np.ascontiguousarray(b1.reshape(64, 1), dtype=f32),
        "b2": np.ascontiguousarray(b2.reshape(32, 1), dtype=f32),
        "b3": np.ascontiguousarray(b3.reshape(1, 1), dtype=f32),
        "bg": np.asarray(bg, dtype=f32).reshape(1, 1),
    }
    # ba shifts all scores by a constant; softmax(s*g) with s+ba needs exp((s+ba)*g).
    # ba == 0 in this problem; assert to be safe.
    assert float(np.asarray(ba)) == 0.0, "nonzero attention bias not supported"
    assert float(np.asarray(bg)) == 0.0, "nonzero AUGRU gate bias not supported"

    in_maps = []
    for c in range(NCORES):
        rows = slice(c * BC, (c + 1) * BC)
        m = dict(common)
        m["user_hist"] = np.ascontiguousarray(user_hist[rows, :Tsteps], dtype=np.int32)
        m["ad_feature"] = np.ascontiguousarray(
            ad_feature[rows].reshape(BC, 1), dtype=np.int32)
        in_maps.append(m)
    return in_maps


_CACHE = {}


def kernel(user_hist, ad_feature, emb, Wih1, Whh1, bih1, bhh1, wa, ba,
           Wih2, Whh2, bih2, bhh2, wg, bg, W1, b1, W2, b2, W3, b3,
           _trace=False, _tsteps=None):
    import concourse.bacc as bacc
    from concourse.bass_utils import run_bass_kernel_spmd

    Tsteps = _tsteps or T
    key = Tsteps
    if key not in _CACHE:
        nc = bacc.Bacc("TRN2", num_devices=1, enable_asserts=True)
        _build(nc, Tsteps)
        nc.compile()
        _CACHE[key] = nc
    nc = _CACHE[key]

    in_maps = _prep_inputs(user_hist, ad_feature, emb, Wih1, Whh1, bih1, bhh1,
                           wa, ba, Wih2, Whh2, bih2, bhh2, wg, bg,
                           W1, b1, W2, b2, W3, b3, Tsteps)
    r = run_bass_kernel_spmd(nc, in_maps, core_ids=list(range(NCORES)),
                             trace=_trace)
    out = np.concatenate(
        [np.asarray(r.results[c]["out"]).reshape(BC, 1) for c in range(NCORES)],
        axis=0)
    if _trace:
        kernel._last_result = r
    return out.astype(np.float32)
